# revision 8
# baseline (speedup 1.0000x reference)
"""ArcFace head on 8 TRN2 NeuronCores (Bass/Tile).

Model-parallel over classes: each of the 8 cores owns a 12500-class slice
of the 100000-class weight matrix and computes its (1024 x 12500) slice of
the logits; the host concatenates slices along the class dim.

Per-core device kernel:
  - normalize embeddings (64/||e|| folded in) and the weight slice
    (1/||w|| folded in), cast to bf16
  - (1024 x 512) @ (512 x 12500) matmul on TensorE, f32 accumulate
  - ArcFace margin: gather the label's weight row per sample
    (indirect DMA), compute cos(theta+m) per row in f32, scatter the
    corrected target logits into the output (indirect DMA, out-of-shard
    rows skipped via the bounds check)

Inputs are pre-arranged on the host (transposed weight slice for the
matmul operands, per-shard relabeled indices); all arithmetic of the op
itself runs on device.
"""

import math

import numpy as np

import concourse.bacc as bacc
import concourse.bass as bass
import concourse.mybir as mybir
import concourse.tile as tile

# Problem constants (hardcoded per harness rules).
B = 1024  # batch
D = 512  # embedding dim
C = 100000  # num classes
NCORES = 8
CS = C // NCORES  # classes per core = 12500
P = 128  # partitions
KCH = D // P  # contraction chunks = 4
NB = B // P  # batch tiles = 8
CW = 500  # class window per matmul (<=512 psum bank, divides 12500)
NCW = CS // CW  # 25 class windows

SCALE = 64.0
MARGIN = 0.5
COS_M = math.cos(MARGIN)
SIN_M = math.sin(MARGIN)
TH = math.cos(math.pi - MARGIN)
MM = math.sin(math.pi - MARGIN) * MARGIN

F32 = mybir.dt.float32
BF16 = mybir.dt.bfloat16
I32 = mybir.dt.int32

OOB_SCATTER = 1 << 26  # out-of-shard sentinel for scatter offsets


def build_graph():
    nc = bacc.Bacc(
        "TRN2",
        target_bir_lowering=False,
        debug=False,
        num_devices=NCORES,
    )

    embT = nc.declare_dram_parameter("embT", [D, B], F32, isOutput=False)
    wT = nc.declare_dram_parameter("wT", [D, CS], F32, isOutput=False)
    w_nat = nc.declare_dram_parameter("w_nat", [CS, D], F32, isOutput=False)
    emb = nc.declare_dram_parameter("emb", [B, D], F32, isOutput=False)
    gidx = nc.declare_dram_parameter("gidx", [P, NB], I32, isOutput=False)
    soff = nc.declare_dram_parameter("soff", [P, NB], I32, isOutput=False)
    out = nc.declare_dram_parameter("out", [B, CS], F32, isOutput=True)

    # DRAM views: partition p of contraction chunk k holds row k*128+p;
    # batch row b maps to (partition b%128, tile b//128).
    embT_r = embT[:].rearrange("(k p) b -> p k b", p=P)  # (128, 4, 1024)
    wT_r = wT[:].rearrange("(k p) c -> p k c", p=P)  # (128, 4, 12500)
    emb_r = emb[:].rearrange("(i p) d -> p i d", p=P)  # (128, 8, 512)
    out_r = out[:].rearrange("(i p) c -> p i c", p=P)  # (128, 8, 12500)
    out_flat = out[:].rearrange("a b -> (a b)")[:, None]  # (12.8M, 1)

    with tile.TileContext(nc) as tc:
        with (
            tc.tile_pool(name="const", bufs=1) as constp,
            tc.tile_pool(name="embp", bufs=1) as embp,
            tc.tile_pool(name="wstage", bufs=3) as wstage,
            tc.tile_pool(name="wsq", bufs=2) as wsqp,
            tc.tile_pool(name="wnt", bufs=3) as wntp,
            tc.tile_pool(name="ostripe", bufs=2) as ostripep,
            tc.tile_pool(name="small", bufs=2) as smallp,
            tc.tile_pool(name="marg", bufs=1) as margp,
            tc.tile_pool(name="ps_main", bufs=4, space="PSUM") as ps_main,
            tc.tile_pool(name="ps_small", bufs=2, space="PSUM") as ps_small,
            tc.tile_pool(name="ps_b", bufs=2, space="PSUM") as ps_b,
        ):
            # Constants.
            ones_col_bf = constp.tile([P, 1], BF16, tag="ones_col")
            nc.vector.memset(ones_col_bf[:], 1.0)
            ones_row_f = constp.tile([1, P], F32, tag="ones_row")
            nc.vector.memset(ones_row_f[:], 1.0)
            s64_row = constp.tile([1, P], F32, tag="s64_row")
            nc.vector.memset(s64_row[:], SCALE)

            # ---------- embedding prep: embT_n = 64 * emb.T / ||emb|| (bf16)
            embT_f = embp.tile([P, KCH, B], F32, tag="embT_f")
            nc.sync.dma_start(out=embT_f[:], in_=embT_r[:])
            emb2 = embp.tile([P, KCH, B], BF16, tag="emb2")
            nc.scalar.square(emb2[:], embT_f[:])
            # ||e||^2 per batch col via ones-matmul (partition reduction).
            eb_ps = []
            for h in range(2):
                pe = ps_small.tile([1, 512], F32, tag="ps_small")
                for k in range(KCH):
                    nc.tensor.matmul(
                        pe[:],
                        lhsT=ones_col_bf[:],
                        rhs=emb2[:, k, h * 512 : (h + 1) * 512],
                        start=(k == 0),
                        stop=(k == KCH - 1),
                    )
                eb_ps.append(pe)
            enorm = smallp.tile([1, B], F32, tag="enorm")
            for h in range(2):
                nc.scalar.sqrt(enorm[:, h * 512 : (h + 1) * 512], eb_ps[h][:])
            erec = smallp.tile([1, B], F32, tag="erec")
            nc.vector.reciprocal(erec[:], enorm[:])
            embT_n = embp.tile([P, KCH, B], BF16, tag="embT_n")
            for h in range(2):
                pb = ps_b.tile([P, 512], F32, tag="ps_b")
                # outer product: 64 * (1/||e||) broadcast to 128 partitions
                nc.tensor.matmul(
                    pb[:],
                    lhsT=s64_row[:],
                    rhs=erec[:, h * 512 : (h + 1) * 512],
                    start=True,
                    stop=True,
                )
                for k in range(KCH):
                    nc.vector.tensor_mul(
                        embT_n[:, k, h * 512 : (h + 1) * 512],
                        embT_f[:, k, h * 512 : (h + 1) * 512],
                        pb[:],
                    )

            # ---------- margin path: corrected target logits per sample
            emb_nat = margp.tile([P, NB, D], F32, tag="emb_nat")
            nc.sync.dma_start(out=emb_nat[:], in_=emb_r[:])
            gidx_t = margp.tile([P, NB], I32, tag="gidx_t")
            nc.sync.dma_start(out=gidx_t[:], in_=gidx[:])
            soff_t = margp.tile([P, NB], I32, tag="soff_t")
            nc.sync.dma_start(out=soff_t[:], in_=soff[:])

            wg = margp.tile([P, NB, D], F32, tag="wg")
            nc.vector.memset(wg[:], 0.0)
            # gather w rows for in-shard labels; out-of-shard rows skipped
            # (one offset per partition per call — the layout the HW
            # indirect DGE path supports)
            for i in range(NB):
                nc.gpsimd.indirect_dma_start(
                    out=wg[:, i, :],
                    out_offset=None,
                    in_=w_nat[:],
                    in_offset=bass.IndirectOffsetOnAxis(
                        ap=gidx_t[:, i : i + 1], axis=0
                    ),
                    bounds_check=CS - 1,
                    oob_is_err=False,
                )

            mtmp = margp.tile([P, NB, D], F32, tag="mtmp")
            en2 = margp.tile([P, NB], F32, tag="en2")
            nc.vector.tensor_mul(mtmp[:], emb_nat[:], emb_nat[:])
            nc.vector.tensor_reduce(
                en2[:], mtmp[:], axis=mybir.AxisListType.X, op=mybir.AluOpType.add
            )
            gn2 = margp.tile([P, NB], F32, tag="gn2")
            nc.vector.tensor_mul(mtmp[:], wg[:], wg[:])
            nc.vector.tensor_reduce(
                gn2[:], mtmp[:], axis=mybir.AxisListType.X, op=mybir.AluOpType.add
            )
            dot = margp.tile([P, NB], F32, tag="dot")
            nc.vector.tensor_mul(mtmp[:], emb_nat[:], wg[:])
            nc.vector.tensor_reduce(
                dot[:], mtmp[:], axis=mybir.AxisListType.X, op=mybir.AluOpType.add
            )
            # cos_t = dot / (||e|| * ||w_label||)
            den = margp.tile([P, NB], F32, tag="den")
            nc.vector.tensor_mul(den[:], en2[:], gn2[:])
            nc.scalar.sqrt(den[:], den[:])
            nc.vector.tensor_scalar_max(den[:], den[:], 1e-12)
            rden = margp.tile([P, NB], F32, tag="rden")
            nc.vector.reciprocal(rden[:], den[:])
            cost = margp.tile([P, NB], F32, tag="cost")
            nc.vector.tensor_mul(cost[:], dot[:], rden[:])
            # sin_t = sqrt(max(0, 1 - cos^2))
            sint = margp.tile([P, NB], F32, tag="sint")
            nc.vector.tensor_mul(sint[:], cost[:], cost[:])
            nc.vector.tensor_scalar(
                out=sint[:],
                in0=sint[:],
                scalar1=-1.0,
                scalar2=1.0,
                op0=mybir.AluOpType.mult,
                op1=mybir.AluOpType.add,
            )
            nc.vector.tensor_scalar_max(sint[:], sint[:], 0.0)
            nc.scalar.sqrt(sint[:], sint[:])
            # cos(t+m) = cos*COS_M - sin*SIN_M ; else branch: cos - MM
            cosm = margp.tile([P, NB], F32, tag="cosm")
            nc.vector.tensor_scalar_mul(cosm[:], sint[:], -SIN_M)
            nc.vector.scalar_tensor_tensor(
                out=cosm[:],
                in0=cost[:],
                scalar=COS_M,
                in1=cosm[:],
                op0=mybir.AluOpType.mult,
                op1=mybir.AluOpType.add,
            )
            alt = margp.tile([P, NB], F32, tag="alt")
            nc.vector.tensor_scalar_add(alt[:], cost[:], -MM)
            mask = margp.tile([P, NB], mybir.dt.uint8, tag="mask")
            nc.vector.tensor_single_scalar(
                mask[:], cost[:], TH, mybir.AluOpType.is_gt
            )
            yv = margp.tile([P, NB], F32, tag="yv")
            nc.vector.select(yv[:], mask[:], cosm[:], alt[:])
            nc.vector.tensor_scalar_mul(yv[:], yv[:], SCALE)

            # ---------- main loop over class windows
            for cw in range(NCW):
                csl = slice(cw * CW, (cw + 1) * CW)
                wt_f = wstage.tile([P, KCH, CW], F32, tag="wt_f")
                nc.sync.dma_start(out=wt_f[:], in_=wT_r[:, :, csl])
                w2 = wsqp.tile([P, KCH, CW], BF16, tag="w2")
                nc.scalar.square(w2[:], wt_f[:])
                pn = ps_small.tile([1, 512], F32, tag="ps_small")
                for k in range(KCH):
                    nc.tensor.matmul(
                        pn[:, :CW],
                        lhsT=ones_col_bf[:],
                        rhs=w2[:, k, :],
                        start=(k == 0),
                        stop=(k == KCH - 1),
                    )
                rn = smallp.tile([1, CW], F32, tag="rn")
                nc.scalar.sqrt(rn[:], pn[:, :CW])
                rrec = smallp.tile([1, CW], F32, tag="rrec")
                nc.vector.reciprocal(rrec[:], rn[:])
                pb = ps_b.tile([P, 512], F32, tag="ps_b")
                nc.tensor.matmul(
                    pb[:, :CW], lhsT=ones_row_f[:], rhs=rrec[:], start=True, stop=True
                )
                wnt = wntp.tile([P, KCH, CW], BF16, tag="wnt")
                for k in range(KCH):
                    nc.vector.tensor_mul(wnt[:, k, :], wt_f[:, k, :], pb[:, :CW])

                ostripe = ostripep.tile([P, NB, CW], F32, tag="ostripe")
                for bt in range(NB):
                    po = ps_main.tile([P, CW], F32, tag="ps_main")
                    for k in range(KCH):
                        nc.tensor.matmul(
                            po[:],
                            lhsT=embT_n[:, k, bt * P : (bt + 1) * P],
                            rhs=wnt[:, k, :],
                            start=(k == 0),
                            stop=(k == KCH - 1),
                        )
                    if bt % 2 == 0:
                        nc.scalar.copy(ostripe[:, bt, :], po[:])
                    else:
                        nc.vector.tensor_copy(ostripe[:, bt, :], po[:])
                nc.sync.dma_start(out=out_r[:, :, csl], in_=ostripe[:])

            # ---------- scatter corrected target logits (after main writes)
            for i in range(NB):
                nc.gpsimd.indirect_dma_start(
                    out=out_flat,
                    out_offset=bass.IndirectOffsetOnAxis(
                        ap=soff_t[:, i : i + 1], axis=0
                    ),
                    in_=yv[:, i : i + 1],
                    in_offset=None,
                    bounds_check=B * CS - 1,
                    oob_is_err=False,
                )

    nc.compile()
    return nc


def make_in_maps(embeddings, labels, weight):
    """Shard + lay out the inputs for the 8 cores."""
    emb = np.ascontiguousarray(embeddings, dtype=np.float32)
    embT = np.ascontiguousarray(emb.T)
    lab = np.asarray(labels).astype(np.int64)
    w = np.asarray(weight, dtype=np.float32)

    bidx = np.arange(B)
    p_of_b = bidx % P  # partition
    i_of_b = bidx // P  # batch tile

    in_maps = []
    for c in range(NCORES):
        lo = c * CS
        local = lab - lo
        in_shard = (local >= 0) & (local < CS)
        gidx = np.full((P, NB), CS, dtype=np.int32)  # CS -> OOB, skipped
        gidx[p_of_b, i_of_b] = np.where(in_shard, local, CS).astype(np.int32)
        soff = np.full((P, NB), OOB_SCATTER, dtype=np.int32)
        soff[p_of_b, i_of_b] = np.where(
            in_shard, bidx * CS + np.clip(local, 0, CS - 1), OOB_SCATTER
        ).astype(np.int32)
        wsh = w[lo : lo + CS]
        in_maps.append(
            {
                "embT": embT,
                "wT": np.ascontiguousarray(wsh.T),
                "w_nat": np.ascontiguousarray(wsh),
                "emb": emb,
                "gidx": gidx,
                "soff": soff,
            }
        )
    return in_maps


_CACHED_NC = None


def _get_graph():
    global _CACHED_NC
    if _CACHED_NC is None:
        _CACHED_NC = build_graph()
    return _CACHED_NC


def kernel(embeddings, labels, weight):
    from concourse.bass_utils import run_bass_kernel_spmd

    nc = _get_graph()
    in_maps = make_in_maps(embeddings, labels, weight)
    res = run_bass_kernel_spmd(nc, in_maps, core_ids=list(range(NCORES)))
    return np.concatenate([res.results[i]["out"] for i in range(NCORES)], axis=1)


if __name__ == "__main__":
    nc = build_graph()
    print("graph built ok")


# revision 13
# speedup vs baseline: 1.0514x; 1.0514x over previous
"""ArcFace head on 8 TRN2 NeuronCores (Bass/Tile).

Model-parallel over classes: each of the 8 cores owns a 12500-class slice
of the 100000-class weight matrix and computes its (1024 x 12500) slice of
the logits; the host concatenates slices along the class dim.

Per-core device kernel:
  - normalize embeddings (64/||e|| folded in) and the weight slice
    (1/||w|| folded in), cast to bf16
  - (1024 x 512) @ (512 x 12500) matmul on TensorE, f32 accumulate
  - ArcFace margin: gather the label's weight row per sample
    (indirect DMA), compute cos(theta+m) per row in f32, scatter the
    corrected target logits into the output (indirect DMA, out-of-shard
    rows skipped via the bounds check)

Inputs are pre-arranged on the host (transposed weight slice for the
matmul operands, per-shard relabeled indices); all arithmetic of the op
itself runs on device.
"""

import math

import numpy as np

import concourse.bacc as bacc
import concourse.bass as bass
import concourse.mybir as mybir
import concourse.tile as tile

# Problem constants (hardcoded per harness rules).
B = 1024  # batch
D = 512  # embedding dim
C = 100000  # num classes
NCORES = 8
CS = C // NCORES  # classes per core = 12500
P = 128  # partitions
KCH = D // P  # contraction chunks = 4
NB = B // P  # batch tiles = 8
CW = 500  # class window per matmul (<=512 psum bank, divides 12500)
NCW = CS // CW  # 25 class windows

SCALE = 64.0
MARGIN = 0.5
COS_M = math.cos(MARGIN)
SIN_M = math.sin(MARGIN)
TH = math.cos(math.pi - MARGIN)
MM = math.sin(math.pi - MARGIN) * MARGIN

F32 = mybir.dt.float32
BF16 = mybir.dt.bfloat16
I32 = mybir.dt.int32

OOB_SCATTER = 1 << 26  # out-of-shard sentinel for scatter offsets


def build_graph():
    nc = bacc.Bacc(
        "TRN2",
        target_bir_lowering=False,
        debug=False,
        num_devices=NCORES,
    )

    embT = nc.declare_dram_parameter("embT", [D, B], F32, isOutput=False)
    wT = nc.declare_dram_parameter("wT", [D, CS], F32, isOutput=False)
    w_nat = nc.declare_dram_parameter("w_nat", [CS, D], F32, isOutput=False)
    emb = nc.declare_dram_parameter("emb", [B, D], F32, isOutput=False)
    gidx = nc.declare_dram_parameter("gidx", [P, NB], I32, isOutput=False)
    soff = nc.declare_dram_parameter("soff", [P, NB], I32, isOutput=False)
    out = nc.declare_dram_parameter("out", [B, CS], F32, isOutput=True)

    # DRAM views: partition p of contraction chunk k holds row k*128+p;
    # batch row b maps to (partition b%128, tile b//128).
    embT_r = embT[:].rearrange("(k p) b -> p k b", p=P)  # (128, 4, 1024)
    wT_r = wT[:].rearrange("(k p) c -> p k c", p=P)  # (128, 4, 12500)
    emb_r = emb[:].rearrange("(i p) d -> p i d", p=P)  # (128, 8, 512)
    out_r = out[:].rearrange("(i p) c -> p i c", p=P)  # (128, 8, 12500)
    out_flat = out[:].rearrange("a b -> (a b)")[:, None]  # (12.8M, 1)

    with tile.TileContext(nc) as tc:
        with (
            tc.tile_pool(name="const", bufs=1) as constp,
            tc.tile_pool(name="embp", bufs=1) as embp,
            tc.tile_pool(name="wstage", bufs=3) as wstage,
            tc.tile_pool(name="wsq", bufs=2) as wsqp,
            tc.tile_pool(name="wnt", bufs=3) as wntp,
            tc.tile_pool(name="ostripe", bufs=2) as ostripep,
            tc.tile_pool(name="small", bufs=2) as smallp,
            tc.tile_pool(name="marg", bufs=1) as margp,
            tc.tile_pool(name="ps_main", bufs=6, space="PSUM") as ps_main,
            tc.tile_pool(name="ps_small", bufs=2, space="PSUM") as ps_small,
        ):
            # Constants.
            ones_col_bf = constp.tile([P, 1], BF16, tag="ones_col")
            nc.vector.memset(ones_col_bf[:], 1.0)

            # ---------- embedding prep: embT_n = 64 * emb.T / ||emb|| (bf16)
            embT_f = embp.tile([P, KCH, B], F32, tag="embT_f")
            nc.sync.dma_start(out=embT_f[:], in_=embT_r[:])
            emb2 = embp.tile([P, KCH, B], BF16, tag="emb2")
            nc.scalar.square(emb2[:], embT_f[:])
            # ||e||^2 per batch col via ones-matmul (partition reduction).
            eb_ps = []
            for h in range(2):
                pe = ps_small.tile([1, 512], F32, tag="ps_small")
                for k in range(KCH):
                    nc.tensor.matmul(
                        pe[:],
                        lhsT=ones_col_bf[:],
                        rhs=emb2[:, k, h * 512 : (h + 1) * 512],
                        start=(k == 0),
                        stop=(k == KCH - 1),
                    )
                eb_ps.append(pe)
            enorm = smallp.tile([1, B], F32, tag="enorm")
            for h in range(2):
                nc.scalar.sqrt(enorm[:, h * 512 : (h + 1) * 512], eb_ps[h][:])
            erec = smallp.tile([1, B], F32, tag="erec")
            escr = smallp.tile([1, B], F32, tag="escr")
            nc.vector.reciprocal_approx_accurate(erec[:], enorm[:], escr[:])
            nc.vector.tensor_scalar_mul(erec[:], erec[:], SCALE)
            ebb = embp.tile([P, B], F32, tag="ebb")
            nc.gpsimd.partition_broadcast(ebb[:], erec[:])
            embT_n = embp.tile([P, KCH, B], BF16, tag="embT_n")
            for k in range(KCH):
                nc.vector.tensor_mul(embT_n[:, k, :], embT_f[:, k, :], ebb[:])

            # ---------- margin path: corrected target logits per sample
            emb_nat = margp.tile([P, NB, D], F32, tag="emb_nat")
            nc.sync.dma_start(out=emb_nat[:], in_=emb_r[:])
            gidx_t = margp.tile([P, NB], I32, tag="gidx_t")
            nc.sync.dma_start(out=gidx_t[:], in_=gidx[:])
            soff_t = margp.tile([P, NB], I32, tag="soff_t")
            nc.sync.dma_start(out=soff_t[:], in_=soff[:])

            wg = margp.tile([P, NB, D], F32, tag="wg")
            nc.vector.memset(wg[:], 0.0)
            # gather w rows for in-shard labels; out-of-shard rows skipped
            # (one offset per partition per call — the layout the HW
            # indirect DGE path supports)
            for i in range(NB):
                nc.gpsimd.indirect_dma_start(
                    out=wg[:, i, :],
                    out_offset=None,
                    in_=w_nat[:],
                    in_offset=bass.IndirectOffsetOnAxis(
                        ap=gidx_t[:, i : i + 1], axis=0
                    ),
                    bounds_check=CS - 1,
                    oob_is_err=False,
                )

            mtmp = margp.tile([P, NB, D], F32, tag="mtmp")
            en2 = margp.tile([P, NB], F32, tag="en2")
            nc.vector.tensor_mul(mtmp[:], emb_nat[:], emb_nat[:])
            nc.vector.tensor_reduce(
                en2[:], mtmp[:], axis=mybir.AxisListType.X, op=mybir.AluOpType.add
            )
            gn2 = margp.tile([P, NB], F32, tag="gn2")
            nc.vector.tensor_mul(mtmp[:], wg[:], wg[:])
            nc.vector.tensor_reduce(
                gn2[:], mtmp[:], axis=mybir.AxisListType.X, op=mybir.AluOpType.add
            )
            dot = margp.tile([P, NB], F32, tag="dot")
            nc.vector.tensor_mul(mtmp[:], emb_nat[:], wg[:])
            nc.vector.tensor_reduce(
                dot[:], mtmp[:], axis=mybir.AxisListType.X, op=mybir.AluOpType.add
            )
            # cos_t = dot / (||e|| * ||w_label||)
            den = margp.tile([P, NB], F32, tag="den")
            nc.vector.tensor_mul(den[:], en2[:], gn2[:])
            nc.scalar.sqrt(den[:], den[:])
            nc.vector.tensor_scalar_max(den[:], den[:], 1e-12)
            rden = margp.tile([P, NB], F32, tag="rden")
            rscr = margp.tile([P, NB], F32, tag="rscr")
            nc.vector.reciprocal_approx_accurate(rden[:], den[:], rscr[:])
            cost = margp.tile([P, NB], F32, tag="cost")
            nc.vector.tensor_mul(cost[:], dot[:], rden[:])
            # sin_t = sqrt(max(0, 1 - cos^2))
            sint = margp.tile([P, NB], F32, tag="sint")
            nc.vector.tensor_mul(sint[:], cost[:], cost[:])
            nc.vector.tensor_scalar(
                out=sint[:],
                in0=sint[:],
                scalar1=-1.0,
                scalar2=1.0,
                op0=mybir.AluOpType.mult,
                op1=mybir.AluOpType.add,
            )
            nc.vector.tensor_scalar_max(sint[:], sint[:], 0.0)
            nc.scalar.sqrt(sint[:], sint[:])
            # cos(t+m) = cos*COS_M - sin*SIN_M ; else branch: cos - MM
            cosm = margp.tile([P, NB], F32, tag="cosm")
            nc.vector.tensor_scalar_mul(cosm[:], sint[:], -SIN_M)
            nc.vector.scalar_tensor_tensor(
                out=cosm[:],
                in0=cost[:],
                scalar=COS_M,
                in1=cosm[:],
                op0=mybir.AluOpType.mult,
                op1=mybir.AluOpType.add,
            )
            alt = margp.tile([P, NB], F32, tag="alt")
            nc.vector.tensor_scalar_add(alt[:], cost[:], -MM)
            mask = margp.tile([P, NB], mybir.dt.uint8, tag="mask")
            nc.vector.tensor_single_scalar(
                mask[:], cost[:], TH, mybir.AluOpType.is_gt
            )
            yv = margp.tile([P, NB], F32, tag="yv")
            nc.vector.select(yv[:], mask[:], cosm[:], alt[:])
            nc.vector.tensor_scalar_mul(yv[:], yv[:], SCALE)

            # ---------- main loop over class windows
            for cw in range(NCW):
                csl = slice(cw * CW, (cw + 1) * CW)
                wt_f = wstage.tile([P, KCH, CW], F32, tag="wt_f")
                nc.sync.dma_start(out=wt_f[:], in_=wT_r[:, :, csl])
                w2 = wsqp.tile([P, KCH, CW], BF16, tag="w2")
                nc.scalar.square(w2[:], wt_f[:])
                pn = ps_small.tile([1, 512], F32, tag="ps_small")
                for k in range(KCH):
                    nc.tensor.matmul(
                        pn[:, :CW],
                        lhsT=ones_col_bf[:],
                        rhs=w2[:, k, :],
                        start=(k == 0),
                        stop=(k == KCH - 1),
                    )
                rn = smallp.tile([1, CW], F32, tag="rn")
                nc.scalar.sqrt(rn[:], pn[:, :CW])
                rrec = smallp.tile([1, CW], F32, tag="rrec")
                rscrw = smallp.tile([1, CW], F32, tag="rscrw")
                nc.vector.reciprocal_approx_accurate(rrec[:], rn[:], rscrw[:])
                wnb = wstage.tile([P, CW], F32, tag="wnb")
                nc.gpsimd.partition_broadcast(wnb[:], rrec[:])
                wnt = wntp.tile([P, KCH, CW], BF16, tag="wnt")
                for k in range(KCH):
                    nc.vector.tensor_mul(wnt[:, k, :], wt_f[:, k, :], wnb[:])

                ostripe = ostripep.tile([P, NB, CW], F32, tag="ostripe")
                for bt in range(NB):
                    po = ps_main.tile([P, CW], F32, tag="ps_main")
                    for k in range(KCH):
                        nc.tensor.matmul(
                            po[:],
                            lhsT=embT_n[:, k, bt * P : (bt + 1) * P],
                            rhs=wnt[:, k, :],
                            start=(k == 0),
                            stop=(k == KCH - 1),
                        )
                    if bt % 2 == 0:
                        nc.scalar.copy(ostripe[:, bt, :], po[:])
                    else:
                        nc.vector.tensor_copy(ostripe[:, bt, :], po[:])
                # out-DMAs on the gpsimd (SWDGE) queue so they never block
                # the sync queue's input prefetch stream
                nc.gpsimd.dma_start(out=out_r[:, :, csl], in_=ostripe[:])

            # ---------- scatter corrected target logits (after main writes)
            for i in range(NB):
                nc.gpsimd.indirect_dma_start(
                    out=out_flat,
                    out_offset=bass.IndirectOffsetOnAxis(
                        ap=soff_t[:, i : i + 1], axis=0
                    ),
                    in_=yv[:, i : i + 1],
                    in_offset=None,
                    bounds_check=B * CS - 1,
                    oob_is_err=False,
                )

    nc.compile()
    return nc


def make_in_maps(embeddings, labels, weight):
    """Shard + lay out the inputs for the 8 cores."""
    emb = np.ascontiguousarray(embeddings, dtype=np.float32)
    embT = np.ascontiguousarray(emb.T)
    lab = np.asarray(labels).astype(np.int64)
    w = np.asarray(weight, dtype=np.float32)

    bidx = np.arange(B)
    p_of_b = bidx % P  # partition
    i_of_b = bidx // P  # batch tile

    in_maps = []
    for c in range(NCORES):
        lo = c * CS
        local = lab - lo
        in_shard = (local >= 0) & (local < CS)
        gidx = np.full((P, NB), CS, dtype=np.int32)  # CS -> OOB, skipped
        gidx[p_of_b, i_of_b] = np.where(in_shard, local, CS).astype(np.int32)
        soff = np.full((P, NB), OOB_SCATTER, dtype=np.int32)
        soff[p_of_b, i_of_b] = np.where(
            in_shard, bidx * CS + np.clip(local, 0, CS - 1), OOB_SCATTER
        ).astype(np.int32)
        wsh = w[lo : lo + CS]
        in_maps.append(
            {
                "embT": embT,
                "wT": np.ascontiguousarray(wsh.T),
                "w_nat": np.ascontiguousarray(wsh),
                "emb": emb,
                "gidx": gidx,
                "soff": soff,
            }
        )
    return in_maps


_CACHED_NC = None


def _get_graph():
    global _CACHED_NC
    if _CACHED_NC is None:
        _CACHED_NC = build_graph()
    return _CACHED_NC


def kernel(embeddings, labels, weight):
    from concourse.bass_utils import run_bass_kernel_spmd

    nc = _get_graph()
    in_maps = make_in_maps(embeddings, labels, weight)
    res = run_bass_kernel_spmd(nc, in_maps, core_ids=list(range(NCORES)))
    return np.concatenate([res.results[i]["out"] for i in range(NCORES)], axis=1)


if __name__ == "__main__":
    nc = build_graph()
    print("graph built ok")


# revision 16
# speedup vs baseline: 1.1441x; 1.0882x over previous
"""ArcFace head on 8 TRN2 NeuronCores (Bass/Tile).

Model-parallel over classes: each of the 8 cores owns a 12500-class slice
of the 100000-class weight matrix and computes its (1024 x 12500) slice of
the logits; the host concatenates slices along the class dim.

Per-core device kernel:
  - normalize embeddings (64/||e|| folded in) and the weight slice
    (1/||w|| folded in), cast to bf16
  - (1024 x 512) @ (512 x 12500) matmul on TensorE, f32 accumulate
  - ArcFace margin: gather the label's weight row per sample
    (indirect DMA), compute cos(theta+m) per row in f32, scatter the
    corrected target logits into the output (indirect DMA, out-of-shard
    rows skipped via the bounds check)

Inputs are pre-arranged on the host (transposed weight slice for the
matmul operands, per-shard relabeled indices); all arithmetic of the op
itself runs on device.
"""

import math

import numpy as np

import concourse.bacc as bacc
import concourse.bass as bass
import concourse.mybir as mybir
import concourse.tile as tile

# Problem constants (hardcoded per harness rules).
B = 1024  # batch
D = 512  # embedding dim
C = 100000  # num classes
NCORES = 8
CS = C // NCORES  # classes per core = 12500
P = 128  # partitions
KCH = D // P  # contraction chunks = 4
NB = B // P  # batch tiles = 8
CW = 500  # class window per matmul (<=512 psum bank, divides 12500)
NCW = CS // CW  # 25 class windows

SCALE = 64.0
MARGIN = 0.5
COS_M = math.cos(MARGIN)
SIN_M = math.sin(MARGIN)
TH = math.cos(math.pi - MARGIN)
MM = math.sin(math.pi - MARGIN) * MARGIN

F32 = mybir.dt.float32
BF16 = mybir.dt.bfloat16
I32 = mybir.dt.int32

OOB_SCATTER = 1 << 26  # out-of-shard sentinel for scatter offsets


def build_graph():
    nc = bacc.Bacc(
        "TRN2",
        target_bir_lowering=False,
        debug=False,
        num_devices=NCORES,
    )

    embT = nc.declare_dram_parameter("embT", [D, B], F32, isOutput=False)
    wT = nc.declare_dram_parameter("wT", [D, CS], F32, isOutput=False)
    w_nat = nc.declare_dram_parameter("w_nat", [CS, D], F32, isOutput=False)
    emb = nc.declare_dram_parameter("emb", [B, D], F32, isOutput=False)
    gidx = nc.declare_dram_parameter("gidx", [P, NB], I32, isOutput=False)
    soff = nc.declare_dram_parameter("soff", [P, NB], I32, isOutput=False)
    out = nc.declare_dram_parameter("out", [B, CS], F32, isOutput=True)

    # DRAM views: partition p of contraction chunk k holds row k*128+p;
    # batch row b maps to (partition b%128, tile b//128).
    embT_r = embT[:].rearrange("(k p) b -> p k b", p=P)  # (128, 4, 1024)
    wT_r = wT[:].rearrange("(k p) c -> p k c", p=P)  # (128, 4, 12500)
    emb_r = emb[:].rearrange("(i p) d -> p i d", p=P)  # (128, 8, 512)
    out_r = out[:].rearrange("(i p) c -> p i c", p=P)  # (128, 8, 12500)
    out_flat = out[:].rearrange("a b -> (a b)")[:, None]  # (12.8M, 1)

    with tile.TileContext(nc) as tc:
        with (
            tc.tile_pool(name="const", bufs=1) as constp,
            tc.tile_pool(name="embp", bufs=1) as embp,
            tc.tile_pool(name="wstage", bufs=4) as wstage,
            tc.tile_pool(name="wnb", bufs=2) as wnbp,
            tc.tile_pool(name="wsq", bufs=2) as wsqp,
            tc.tile_pool(name="wnt", bufs=3) as wntp,
            tc.tile_pool(name="ostripe", bufs=2) as ostripep,
            tc.tile_pool(name="small", bufs=2) as smallp,
            tc.tile_pool(name="marg", bufs=1) as margp,
            tc.tile_pool(name="ps_main", bufs=6, space="PSUM") as ps_main,
            tc.tile_pool(name="ps_small", bufs=2, space="PSUM") as ps_small,
        ):
            # Constants.
            ones_col_bf = constp.tile([P, 1], BF16, tag="ones_col")
            nc.vector.memset(ones_col_bf[:], 1.0)

            # ---------- embedding prep: embT_n = 64 * emb.T / ||emb|| (bf16)
            embT_f = embp.tile([P, KCH, B], F32, tag="embT_f")
            nc.sync.dma_start(out=embT_f[:], in_=embT_r[:])
            emb2 = embp.tile([P, KCH, B], BF16, tag="emb2")
            nc.scalar.square(emb2[:], embT_f[:])
            # ||e||^2 per batch col via ones-matmul (partition reduction).
            eb_ps = []
            for h in range(2):
                pe = ps_small.tile([1, 512], F32, tag="ps_small")
                for k in range(KCH):
                    nc.tensor.matmul(
                        pe[:],
                        lhsT=ones_col_bf[:],
                        rhs=emb2[:, k, h * 512 : (h + 1) * 512],
                        start=(k == 0),
                        stop=(k == KCH - 1),
                    )
                eb_ps.append(pe)
            enorm = smallp.tile([1, B], F32, tag="enorm")
            for h in range(2):
                nc.scalar.sqrt(enorm[:, h * 512 : (h + 1) * 512], eb_ps[h][:])
            erec = smallp.tile([1, B], F32, tag="erec")
            escr = smallp.tile([1, B], F32, tag="escr")
            nc.vector.reciprocal_approx_accurate(erec[:], enorm[:], escr[:])
            nc.vector.tensor_scalar_mul(erec[:], erec[:], SCALE)
            ebb = embp.tile([P, B], F32, tag="ebb")
            nc.gpsimd.partition_broadcast(ebb[:], erec[:])
            embT_n = embp.tile([P, KCH, B], BF16, tag="embT_n")
            for k in range(KCH):
                nc.vector.tensor_mul(embT_n[:, k, :], embT_f[:, k, :], ebb[:])

            # ---------- margin path: corrected target logits per sample
            emb_nat = margp.tile([P, NB, D], F32, tag="emb_nat")
            nc.sync.dma_start(out=emb_nat[:], in_=emb_r[:])
            gidx_t = margp.tile([P, NB], I32, tag="gidx_t")
            nc.sync.dma_start(out=gidx_t[:], in_=gidx[:])
            soff_t = margp.tile([P, NB], I32, tag="soff_t")
            nc.sync.dma_start(out=soff_t[:], in_=soff[:])

            wg = margp.tile([P, NB, D], F32, tag="wg")
            nc.vector.memset(wg[:], 0.0)
            # gather w rows for in-shard labels; out-of-shard rows skipped
            # (one offset per partition per call — the layout the HW
            # indirect DGE path supports)
            for i in range(NB):
                nc.gpsimd.indirect_dma_start(
                    out=wg[:, i, :],
                    out_offset=None,
                    in_=w_nat[:],
                    in_offset=bass.IndirectOffsetOnAxis(
                        ap=gidx_t[:, i : i + 1], axis=0
                    ),
                    bounds_check=CS - 1,
                    oob_is_err=False,
                )

            mtmp = margp.tile([P, NB, D], F32, tag="mtmp")
            en2 = margp.tile([P, NB], F32, tag="en2")
            nc.vector.tensor_mul(mtmp[:], emb_nat[:], emb_nat[:])
            nc.vector.tensor_reduce(
                en2[:], mtmp[:], axis=mybir.AxisListType.X, op=mybir.AluOpType.add
            )
            gn2 = margp.tile([P, NB], F32, tag="gn2")
            nc.vector.tensor_mul(mtmp[:], wg[:], wg[:])
            nc.vector.tensor_reduce(
                gn2[:], mtmp[:], axis=mybir.AxisListType.X, op=mybir.AluOpType.add
            )
            dot = margp.tile([P, NB], F32, tag="dot")
            nc.vector.tensor_mul(mtmp[:], emb_nat[:], wg[:])
            nc.vector.tensor_reduce(
                dot[:], mtmp[:], axis=mybir.AxisListType.X, op=mybir.AluOpType.add
            )
            # cos_t = dot / (||e|| * ||w_label||)
            den = margp.tile([P, NB], F32, tag="den")
            nc.vector.tensor_mul(den[:], en2[:], gn2[:])
            nc.scalar.sqrt(den[:], den[:])
            nc.vector.tensor_scalar_max(den[:], den[:], 1e-12)
            rden = margp.tile([P, NB], F32, tag="rden")
            rscr = margp.tile([P, NB], F32, tag="rscr")
            nc.vector.reciprocal_approx_accurate(rden[:], den[:], rscr[:])
            cost = margp.tile([P, NB], F32, tag="cost")
            nc.vector.tensor_mul(cost[:], dot[:], rden[:])
            # sin_t = sqrt(max(0, 1 - cos^2))
            sint = margp.tile([P, NB], F32, tag="sint")
            nc.vector.tensor_mul(sint[:], cost[:], cost[:])
            nc.vector.tensor_scalar(
                out=sint[:],
                in0=sint[:],
                scalar1=-1.0,
                scalar2=1.0,
                op0=mybir.AluOpType.mult,
                op1=mybir.AluOpType.add,
            )
            nc.vector.tensor_scalar_max(sint[:], sint[:], 0.0)
            nc.scalar.sqrt(sint[:], sint[:])
            # cos(t+m) = cos*COS_M - sin*SIN_M ; else branch: cos - MM
            cosm = margp.tile([P, NB], F32, tag="cosm")
            nc.vector.tensor_scalar_mul(cosm[:], sint[:], -SIN_M)
            nc.vector.scalar_tensor_tensor(
                out=cosm[:],
                in0=cost[:],
                scalar=COS_M,
                in1=cosm[:],
                op0=mybir.AluOpType.mult,
                op1=mybir.AluOpType.add,
            )
            alt = margp.tile([P, NB], F32, tag="alt")
            nc.vector.tensor_scalar_add(alt[:], cost[:], -MM)
            mask = margp.tile([P, NB], mybir.dt.uint8, tag="mask")
            nc.vector.tensor_single_scalar(
                mask[:], cost[:], TH, mybir.AluOpType.is_gt
            )
            yv = margp.tile([P, NB], F32, tag="yv")
            nc.vector.select(yv[:], mask[:], cosm[:], alt[:])
            nc.vector.tensor_scalar_mul(yv[:], yv[:], SCALE)

            # ---------- main loop over class windows, weight prep pipelined
            # one iteration ahead of the matmuls that consume it
            def prep(cw):
                csl = slice(cw * CW, (cw + 1) * CW)
                wt_f = wstage.tile([P, KCH, CW], F32, tag="wt_f")
                nc.sync.dma_start(out=wt_f[:], in_=wT_r[:, :, csl])
                w2 = wsqp.tile([P, KCH, CW], BF16, tag="w2")
                nc.scalar.square(w2[:], wt_f[:])
                pn = ps_small.tile([1, 512], F32, tag="ps_small")
                for k in range(KCH):
                    nc.tensor.matmul(
                        pn[:, :CW],
                        lhsT=ones_col_bf[:],
                        rhs=w2[:, k, :],
                        start=(k == 0),
                        stop=(k == KCH - 1),
                    )
                rn = smallp.tile([1, CW], F32, tag="rn")
                nc.scalar.sqrt(rn[:], pn[:, :CW])
                rrec = smallp.tile([1, CW], F32, tag="rrec")
                rscrw = smallp.tile([1, CW], F32, tag="rscrw")
                nc.vector.reciprocal_approx_accurate(rrec[:], rn[:], rscrw[:])
                wnb = wnbp.tile([P, CW], F32, tag="wnb")
                nc.gpsimd.partition_broadcast(wnb[:], rrec[:])
                wnt = wntp.tile([P, KCH, CW], BF16, tag="wnt")
                for k in range(KCH):
                    nc.vector.tensor_mul(wnt[:, k, :], wt_f[:, k, :], wnb[:])
                return wnt

            wnt_cur = prep(0)
            for cw in range(NCW):
                wnt_next = prep(cw + 1) if cw + 1 < NCW else None
                ostripe = ostripep.tile([P, NB, CW], F32, tag="ostripe")
                for bt in range(NB):
                    po = ps_main.tile([P, CW], F32, tag="ps_main")
                    for k in range(KCH):
                        nc.tensor.matmul(
                            po[:],
                            lhsT=embT_n[:, k, bt * P : (bt + 1) * P],
                            rhs=wnt_cur[:, k, :],
                            start=(k == 0),
                            stop=(k == KCH - 1),
                        )
                    if bt % 2 == 0:
                        nc.scalar.copy(ostripe[:, bt, :], po[:])
                    else:
                        nc.vector.tensor_copy(ostripe[:, bt, :], po[:])
                # out-DMAs on the gpsimd (SWDGE) queue so they never block
                # the sync queue's input prefetch stream
                nc.gpsimd.dma_start(
                    out=out_r[:, :, cw * CW : (cw + 1) * CW], in_=ostripe[:]
                )
                wnt_cur = wnt_next

            # ---------- scatter corrected target logits (after main writes)
            for i in range(NB):
                nc.gpsimd.indirect_dma_start(
                    out=out_flat,
                    out_offset=bass.IndirectOffsetOnAxis(
                        ap=soff_t[:, i : i + 1], axis=0
                    ),
                    in_=yv[:, i : i + 1],
                    in_offset=None,
                    bounds_check=B * CS - 1,
                    oob_is_err=False,
                )

    nc.compile()
    return nc


def make_in_maps(embeddings, labels, weight):
    """Shard + lay out the inputs for the 8 cores."""
    emb = np.ascontiguousarray(embeddings, dtype=np.float32)
    embT = np.ascontiguousarray(emb.T)
    lab = np.asarray(labels).astype(np.int64)
    w = np.asarray(weight, dtype=np.float32)

    bidx = np.arange(B)
    p_of_b = bidx % P  # partition
    i_of_b = bidx // P  # batch tile

    in_maps = []
    for c in range(NCORES):
        lo = c * CS
        local = lab - lo
        in_shard = (local >= 0) & (local < CS)
        gidx = np.full((P, NB), CS, dtype=np.int32)  # CS -> OOB, skipped
        gidx[p_of_b, i_of_b] = np.where(in_shard, local, CS).astype(np.int32)
        soff = np.full((P, NB), OOB_SCATTER, dtype=np.int32)
        soff[p_of_b, i_of_b] = np.where(
            in_shard, bidx * CS + np.clip(local, 0, CS - 1), OOB_SCATTER
        ).astype(np.int32)
        wsh = w[lo : lo + CS]
        in_maps.append(
            {
                "embT": embT,
                "wT": np.ascontiguousarray(wsh.T),
                "w_nat": np.ascontiguousarray(wsh),
                "emb": emb,
                "gidx": gidx,
                "soff": soff,
            }
        )
    return in_maps


_CACHED_NC = None


def _get_graph():
    global _CACHED_NC
    if _CACHED_NC is None:
        _CACHED_NC = build_graph()
    return _CACHED_NC


def kernel(embeddings, labels, weight):
    from concourse.bass_utils import run_bass_kernel_spmd

    nc = _get_graph()
    in_maps = make_in_maps(embeddings, labels, weight)
    res = run_bass_kernel_spmd(nc, in_maps, core_ids=list(range(NCORES)))
    return np.concatenate([res.results[i]["out"] for i in range(NCORES)], axis=1)


if __name__ == "__main__":
    nc = build_graph()
    print("graph built ok")
